# revision 21
# baseline (speedup 1.0000x reference)
"""Trainium2 Bass kernel for the 2-class Gaussian prior log-prob loss.

Reference math (N=8192 samples, D=3072 dims, 2 balanced classes):
    mus[c]  = mean over rows of class c of `mean`
    lsds[c] = mean over rows of class c of `log_sd`
    logp[n] = sum_d [ -0.5*log(2pi) - lsds[t_n,d]
                      - 0.5*(z[n,d]-mus[t_n,d])^2 * exp(-2*lsds[t_n,d]) ]
    log_p_total[c] = class-mean of logp;  prior = mean(log_p_total + logdet_total)

Distribution: COLUMN-parallel — core i owns d-slice [i*384, (i+1)*384) of all
8192 rows. The per-class mean reduction runs over N, which is fully local in
this layout, so no collective is needed at all: each core produces its slice
of mus/lsds exactly, plus a per-sample partial
    rq_i[n] = sum_{d in slice_i} ((z[n,d]-mus[t_n,d]) * sqrt(0.5*e^-2*lsds))^2
and the host adds the 8 partial vectors (8 x [8192] floats), applies
    logp[n] = C2[t_n] - sum_i rq_i[n],  C2[c] = -0.5*log(2pi)*D - sum_d lsds[c,d]
and finishes the tiny class-level reductions.

On-core schedule: mean/log_sd stream in as [128, 2*4*384] packed tiles (4
row-blocks side by side, mean|log_sd concatenated) and are position-wise
summed on the VectorEngine; a one-hot matmul turns the folded [128, 768]
accumulator into per-class sums. Phase 2 runs in place on packed z tiles:
DVE subtract/multiply, then ScalarE Square with per-row-block accumulate.
"""

import numpy as np

import concourse.bass as bass
import concourse.bacc as bacc
import concourse.mybir as mybir
import concourse.tile as tile
from concourse.bass_utils import run_bass_kernel_spmd

LOG_2PI = float(np.log(2.0 * np.pi))

N_CORES = 8
N = 8192
D = 3072
W = D // N_CORES           # columns per core (384)
P = 128                    # SBUF partitions
B = 4                      # row-blocks packed per SBUF tile
RT = N // P                # 64 row blocks
T = RT // B                # 16 packed tiles
PW = B * W                 # packed tile width (1536)

F32 = mybir.dt.float32
BF16 = mybir.dt.bfloat16

# Set by test.py to capture a hardware trace; last BassKernelResults lands in
# LAST_RESULTS for exec-time inspection.
TRACE = False
LAST_RESULTS = None

_CACHED_NC = None


def _build_nc():
    nc = bacc.Bacc(
        "TRN2",
        target_bir_lowering=False,
        debug=False,
        num_devices=N_CORES,
    )

    # host-packed layouts: row (t*P + p), so each tile DMA is contiguous.
    # ml = [mean-blocks | lsd-blocks] side by side; z is bf16.
    z_d = nc.dram_tensor("z", [T * P, PW], BF16, kind="ExternalInput").ap()
    ml_d = nc.dram_tensor("ml", [T * P, 2 * PW], F32, kind="ExternalInput").ap()
    oh_d = nc.dram_tensor("oh", [P, 2], F32, kind="ExternalInput").ap()
    selT_d = nc.dram_tensor("selT", [2, P], F32, kind="ExternalInput").ap()
    invc_d = nc.dram_tensor("invc", [2, 1], F32, kind="ExternalInput").ap()

    rq_d = nc.dram_tensor("rq", [P, RT], F32, kind="ExternalOutput").ap()
    mus_d = nc.dram_tensor("mus", [2, W], F32, kind="ExternalOutput").ap()
    lsds_d = nc.dram_tensor("lsds", [2, W], F32, kind="ExternalOutput").ap()

    with tile.TileContext(nc) as tc:
        with (
            tc.tile_pool(name="consts", bufs=1) as cp,
            tc.tile_pool(name="stream", bufs=5) as sp,
            tc.tile_pool(name="zpool", bufs=16) as zp,
            tc.tile_pool(name="coef", bufs=2) as cfp,
            tc.tile_pool(name="wsc", bufs=2) as wsc,
            tc.tile_pool(name="acc", bufs=1, space="PSUM") as pp,
            tc.tile_pool(name="bcast", bufs=2, space="PSUM") as pbp,
        ):
            oh = cp.tile([P, 2], F32)
            nc.scalar.dma_start(oh, oh_d)
            selT = cp.tile([2, P], F32)
            nc.scalar.dma_start(selT, selT_d)
            invc = cp.tile([2, 1], F32)
            nc.scalar.dma_start(invc, invc_d)

            # ---- phase 1: position-wise sum of all row blocks ----
            # 16-deep position-wise accumulate chain (keeps fp32 error low);
            # the B-block fold happens inside the PSUM matmul accumulation.
            acc = cp.tile([P, 2 * PW], F32)
            nc.gpsimd.memset(acc, 0.0)
            for t in range(T):
                mlt = sp.tile([P, 2 * PW], F32, tag="ml", name=f"ml{t}")
                nc.sync.dma_start(mlt, ml_d[t * P:(t + 1) * P, :])
                nc.vector.tensor_add(acc, acc, mlt)

            # one-hot matmuls: per-class sums, PSUM-accumulated over the B
            # blocks. Each output sits in its own PSUM bank (512 fp32).
            accv = acc.rearrange("p (s b w) -> p s b w", s=2, b=B)
            ps = pp.tile([2, 1024], F32)
            for b in range(B):
                nc.tensor.matmul(
                    ps[:, 0:W], oh, accv[:, 0, b, :],
                    start=(b == 0), stop=(b == B - 1),
                )
            for b in range(B):
                nc.tensor.matmul(
                    ps[:, 512:512 + W], oh, accv[:, 1, b, :],
                    start=(b == 0), stop=(b == B - 1),
                )

            # ---- coefficients (all local: no collective needed) ----
            M = cfp.tile([2, 2 * W], F32, tag="c", name="M")
            psv = ps.rearrange("c (s x) -> c s x", s=2)[:, :, 0:W]
            nc.vector.tensor_scalar_mul(
                M.rearrange("c (s w) -> c s w", s=2), psv, invc
            )  # [mu | lsd] slice means
            nc.scalar.dma_start(mus_d, M[:, 0:W])
            nc.scalar.dma_start(lsds_d, M[:, W:2 * W])
            # sqrt(0.5*exp(-2*lsd)) == exp(-lsd + 0.5*ln(0.5)): one ACT op
            bln = cp.tile([2, 1], F32)
            nc.gpsimd.memset(bln, float(0.5 * np.log(0.5)))
            SBc = cfp.tile([2, W], F32, tag="c2", name="SBc")
            nc.scalar.activation(
                SBc, M[:, W:2 * W], mybir.ActivationFunctionType.Exp,
                bias=bln[:, 0:1], scale=-1.0,
            )

            # Broadcast class rows to the 128-row pattern, replicated B times
            # (bf16, matching the z tiles): MU4/SB4[p, b*W+w] = coef[class(p), w]
            MU4 = cp.tile([P, PW], BF16)
            SB4 = cp.tile([P, PW], BF16)
            pmu = pbp.tile([P, W], F32, tag="bc", name="pmu")
            nc.tensor.matmul(pmu, selT, M[:, 0:W], start=True, stop=True)
            psb = pbp.tile([P, W], F32, tag="bc", name="psb")
            nc.tensor.matmul(psb, selT, SBc, start=True, stop=True)
            for b in range(B):
                cs = slice(b * W, (b + 1) * W)
                nc.scalar.copy(MU4[:, cs], pmu)
                nc.vector.tensor_copy(SB4[:, cs], psb)

            # ---- phase 2: rq partials, in place on packed bf16 z tiles ----
            # Work is split across three engine lanes so no single 64-op
            # reduce chain paces the loop: tiles 0..7 reduce on ScalarE
            # (Square+accum), 8..11 entirely on VectorE, 12..15 entirely on
            # GpSimd.
            rq_sb = cp.tile([P, RT], F32)
            N_ACT, N_DVE = 9, 3
            for t in range(T):
                zt = zp.tile([P, PW], BF16, tag="z", name=f"z{t}")
                # same HWDGE ring as the mean/log_sd stream: ring FIFO keeps
                # the phase-1 stream strictly ahead of the z prefetch
                nc.sync.dma_start(zt, z_d[t * P:(t + 1) * P, :])
                eng = nc.vector if t < N_ACT + N_DVE else nc.gpsimd
                eng.tensor_sub(zt, zt, MU4)
                eng.tensor_mul(zt, zt, SB4)
                for b in range(B):
                    cs = slice(b * W, (b + 1) * W)
                    ro = rq_sb[:, t * B + b:t * B + b + 1]
                    if t < N_ACT:
                        nc.scalar.activation(
                            zt[:, cs], zt[:, cs],
                            mybir.ActivationFunctionType.Square,
                            accum_out=ro,
                        )
                    else:
                        # TensorScalarPtr doesn't codegen on Pool: reduces
                        # always run on VectorE
                        vsq = wsc.tile([P, W], F32, tag="vsq", name=f"vs{t}_{b}")
                        nc.vector.scalar_tensor_tensor(
                            out=vsq, in0=zt[:, cs], scalar=1.0, in1=zt[:, cs],
                            op0=mybir.AluOpType.mult,
                            op1=mybir.AluOpType.mult, accum_out=ro,
                        )
            nc.scalar.dma_start(rq_d, rq_sb)

    nc.compile()
    return nc


def kernel(z, mean, log_sd, logdet, target):
    global LAST_RESULTS, _CACHED_NC

    z = np.asarray(z, dtype=np.float32)
    mean = np.asarray(mean, dtype=np.float32)
    log_sd = np.asarray(log_sd, dtype=np.float32)
    logdet64 = np.asarray(logdet, dtype=np.float64)
    tgt = np.asarray(target).astype(np.int64)
    n, d = z.shape
    assert (n, d) == (N, D), f"kernel hardcoded for {(N, D)}, got {(n, d)}"

    # The device kernel assumes every 128-row block has the same class pattern
    # (true for the arange%2 labels). Otherwise interleave the (balanced)
    # classes host-side and un-permute logp afterwards.
    pat = tgt[:P]
    perm = None
    tgt_dev = tgt
    if not (tgt.reshape(-1, P) == pat[None, :]).all():
        idx0 = np.where(tgt == 0)[0]
        idx1 = np.where(tgt == 1)[0]
        assert len(idx0) == len(idx1), "fallback layout needs balanced classes"
        perm = np.empty(n, dtype=np.int64)
        perm[0::2] = idx0
        perm[1::2] = idx1
        z, mean, log_sd = z[perm], mean[perm], log_sd[perm]
        tgt_dev = tgt[perm]
        pat = tgt_dev[:P]

    counts = np.array([(tgt == 0).sum(), (tgt == 1).sum()], dtype=np.float64)
    patf = pat.astype(np.float32)
    oh_np = np.ascontiguousarray(np.stack([1.0 - patf, patf], axis=1))  # [P, 2]
    selT_np = np.ascontiguousarray(oh_np.T)  # [2, P]
    invc_np = (1.0 / counts).astype(np.float32).reshape(2, 1)

    if _CACHED_NC is None:
        _CACHED_NC = _build_nc()
    nc = _CACHED_NC

    bf16 = mybir.dt.np(BF16)

    def pack(x, i, dtype):
        # [N, W] slice -> [T*P, B*W]: row (t*P+p), block b at cols [b*W,(b+1)*W)
        s = x[:, i * W:(i + 1) * W].reshape(T, B, P, W)
        return np.ascontiguousarray(
            s.transpose(0, 2, 1, 3).reshape(T * P, B * W).astype(dtype)
        )

    in_maps = [
        {
            "z": pack(z, i, bf16),
            "ml": np.concatenate(
                [pack(mean, i, np.float32), pack(log_sd, i, np.float32)], axis=1
            ),
            "oh": oh_np,
            "selT": selT_np,
            "invc": invc_np,
        }
        for i in range(N_CORES)
    ]
    res = run_bass_kernel_spmd(
        nc, in_maps, core_ids=list(range(N_CORES)), trace=TRACE
    )
    LAST_RESULTS = res

    # rq[p, b] holds the partial for sample n = b*128 + p on each core
    rq = np.zeros(N, dtype=np.float64)
    for i in range(N_CORES):
        rq += np.asarray(res.results[i]["rq"]).T.reshape(-1).astype(np.float64)
    mus = np.concatenate(
        [np.asarray(res.results[i]["mus"]) for i in range(N_CORES)], axis=1
    )
    lsds = np.concatenate(
        [np.asarray(res.results[i]["lsds"]) for i in range(N_CORES)], axis=1
    )

    # logp[n] = C2[t_n] - rq[n];  C2[c] = -0.5*log(2pi)*D - sum_d lsds[c, d]
    c2 = -0.5 * LOG_2PI * D - lsds.astype(np.float64).sum(axis=1)
    logp = (c2[tgt_dev] - rq).astype(np.float32)
    if perm is not None:
        inv = np.empty_like(perm)
        inv[perm] = np.arange(n)
        logp = logp[inv]

    logp64 = logp.astype(np.float64)
    lp_tot = np.array(
        [logp64[tgt == 0].sum() / counts[0], logp64[tgt == 1].sum() / counts[1]]
    )
    ld_tot = np.array(
        [logdet64[tgt == 0].sum() / counts[0], logdet64[tgt == 1].sum() / counts[1]]
    )
    prior_logprob = np.float32((lp_tot + ld_tot).mean())
    log_p_total = lp_tot.astype(np.float32)

    return prior_logprob, mus, lsds, logp, log_p_total


# revision 22
# speedup vs baseline: 1.2350x; 1.2350x over previous
"""Trainium2 Bass kernel for the 2-class Gaussian prior log-prob loss.

Reference math (N=8192 samples, D=3072 dims, 2 balanced classes):
    mus[c]  = mean over rows of class c of `mean`
    lsds[c] = mean over rows of class c of `log_sd`
    logp[n] = sum_d [ -0.5*log(2pi) - lsds[t_n,d]
                      - 0.5*(z[n,d]-mus[t_n,d])^2 * exp(-2*lsds[t_n,d]) ]
    log_p_total[c] = class-mean of logp;  prior = mean(log_p_total + logdet_total)

Distribution: COLUMN-parallel — core i owns d-slice [i*384, (i+1)*384) of all
8192 rows. The per-class mean reduction runs over N, which is fully local in
this layout, so no collective is needed at all: each core produces its slice
of mus/lsds exactly, plus a per-sample partial
    rq_i[n] = sum_{d in slice_i} ((z[n,d]-mus[t_n,d]) * sqrt(0.5*e^-2*lsds))^2
and the host adds the 8 partial vectors (8 x [8192] floats), applies
    logp[n] = C2[t_n] - sum_i rq_i[n],  C2[c] = -0.5*log(2pi)*D - sum_d lsds[c,d]
and finishes the tiny class-level reductions.

On-core schedule: mean/log_sd stream in as [128, 2*4*384] packed tiles (4
row-blocks side by side, mean|log_sd concatenated) and are position-wise
summed on the VectorEngine; a one-hot matmul turns the folded [128, 768]
accumulator into per-class sums. Phase 2 runs in place on packed z tiles:
DVE subtract/multiply, then ScalarE Square with per-row-block accumulate.
"""

import numpy as np

import concourse.bass as bass
import concourse.bacc as bacc
import concourse.mybir as mybir
import concourse.tile as tile
from concourse.bass_utils import run_bass_kernel_spmd

LOG_2PI = float(np.log(2.0 * np.pi))

N_CORES = 8
N = 8192
D = 3072
W = D // N_CORES           # columns per core (384)
P = 128                    # SBUF partitions
B = 4                      # row-blocks packed per SBUF tile
RT = N // P                # 64 row blocks
T = RT // B                # 16 packed tiles
PW = B * W                 # packed tile width (1536)

F32 = mybir.dt.float32
BF16 = mybir.dt.bfloat16

# Set by test.py to capture a hardware trace; last BassKernelResults lands in
# LAST_RESULTS for exec-time inspection.
TRACE = False
LAST_RESULTS = None

_CACHED_NC = None


def _build_nc():
    nc = bacc.Bacc(
        "TRN2",
        target_bir_lowering=False,
        debug=False,
        num_devices=N_CORES,
    )

    # host-packed layouts: row (t*P + p), so each tile DMA is contiguous.
    # ml = [mean-blocks | lsd-blocks] side by side; z is bf16.
    z_d = nc.dram_tensor("z", [T * P, PW], BF16, kind="ExternalInput").ap()
    ml_d = nc.dram_tensor("ml", [T * P, 2 * PW], F32, kind="ExternalInput").ap()
    oh_d = nc.dram_tensor("oh", [P, 2], F32, kind="ExternalInput").ap()
    selT_d = nc.dram_tensor("selT", [2, P], F32, kind="ExternalInput").ap()
    invc_d = nc.dram_tensor("invc", [2, 1], F32, kind="ExternalInput").ap()

    rq_d = nc.dram_tensor("rq", [P, RT], F32, kind="ExternalOutput").ap()
    mus_d = nc.dram_tensor("mus", [2, W], F32, kind="ExternalOutput").ap()
    lsds_d = nc.dram_tensor("lsds", [2, W], F32, kind="ExternalOutput").ap()

    with tile.TileContext(nc) as tc:
        with (
            tc.tile_pool(name="consts", bufs=1) as cp,
            tc.tile_pool(name="stream", bufs=5) as sp,
            tc.tile_pool(name="zpool", bufs=16) as zp,
            tc.tile_pool(name="coef", bufs=2) as cfp,
            tc.tile_pool(name="wsc", bufs=2) as wsc,
            tc.tile_pool(name="acc", bufs=1, space="PSUM") as pp,
            tc.tile_pool(name="bcast", bufs=2, space="PSUM") as pbp,
        ):
            oh = cp.tile([P, 2], F32)
            nc.scalar.dma_start(oh, oh_d)
            selT = cp.tile([2, P], F32)
            nc.scalar.dma_start(selT, selT_d)
            invc = cp.tile([2, 1], F32)
            nc.scalar.dma_start(invc, invc_d)

            # ---- phase 1: position-wise sum of all row blocks ----
            # 16-deep position-wise accumulate chain (keeps fp32 error low);
            # the B-block fold happens inside the PSUM matmul accumulation.
            acc = cp.tile([P, 2 * PW], F32)
            nc.gpsimd.memset(acc, 0.0)
            for t in range(T):
                mlt = sp.tile([P, 2 * PW], F32, tag="ml", name=f"ml{t}")
                nc.sync.dma_start(mlt, ml_d[t * P:(t + 1) * P, :])
                nc.vector.tensor_add(acc, acc, mlt)

            # one-hot matmuls: per-class sums, PSUM-accumulated over the B
            # blocks. Each output sits in its own PSUM bank (512 fp32).
            accv = acc.rearrange("p (s b w) -> p s b w", s=2, b=B)
            ps = pp.tile([2, 1024], F32)
            for b in range(B):
                nc.tensor.matmul(
                    ps[:, 0:W], oh, accv[:, 0, b, :],
                    start=(b == 0), stop=(b == B - 1),
                )
            for b in range(B):
                nc.tensor.matmul(
                    ps[:, 512:512 + W], oh, accv[:, 1, b, :],
                    start=(b == 0), stop=(b == B - 1),
                )

            # ---- coefficients (all local: no collective needed) ----
            M = cfp.tile([2, 2 * W], F32, tag="c", name="M")
            psv = ps.rearrange("c (s x) -> c s x", s=2)[:, :, 0:W]
            nc.vector.tensor_scalar_mul(
                M.rearrange("c (s w) -> c s w", s=2), psv, invc
            )  # [mu | lsd] slice means
            nc.scalar.dma_start(mus_d, M[:, 0:W])
            nc.scalar.dma_start(lsds_d, M[:, W:2 * W])
            # sqrt(0.5*exp(-2*lsd)) == exp(-lsd + 0.5*ln(0.5)): one ACT op
            bln = cp.tile([2, 1], F32)
            nc.gpsimd.memset(bln, float(0.5 * np.log(0.5)))
            SBc = cfp.tile([2, W], F32, tag="c2", name="SBc")
            nc.scalar.activation(
                SBc, M[:, W:2 * W], mybir.ActivationFunctionType.Exp,
                bias=bln[:, 0:1], scale=-1.0,
            )

            # Broadcast class rows to the 128-row pattern, replicated B times
            # (bf16, matching the z tiles): MU4/SB4[p, b*W+w] = coef[class(p), w]
            MU4 = cp.tile([P, PW], BF16)
            SB4 = cp.tile([P, PW], BF16)
            pmu = pbp.tile([P, W], F32, tag="bc", name="pmu")
            nc.tensor.matmul(pmu, selT, M[:, 0:W], start=True, stop=True)
            psb = pbp.tile([P, W], F32, tag="bc", name="psb")
            nc.tensor.matmul(psb, selT, SBc, start=True, stop=True)
            for b in range(B):
                cs = slice(b * W, (b + 1) * W)
                nc.scalar.copy(MU4[:, cs], pmu)
                nc.vector.tensor_copy(SB4[:, cs], psb)

            # ---- phase 2: rq partials, in place on packed bf16 z tiles ----
            # The per-block reduce is split between ScalarE (Square+accum,
            # leading tiles) and VectorE (scalar_tensor_tensor v*v with
            # accumulate, trailing tiles) so neither 64-op chain paces the
            # loop. GpSimd offload was tried and regressed (DVE/POOL shared
            # SBUF port contention).
            rq_sb = cp.tile([P, RT], F32)
            N_ACT = 9
            for t in range(T):
                zt = zp.tile([P, PW], BF16, tag="z", name=f"z{t}")
                # same HWDGE ring as the mean/log_sd stream: ring FIFO keeps
                # the phase-1 stream strictly ahead of the z prefetch
                nc.sync.dma_start(zt, z_d[t * P:(t + 1) * P, :])
                nc.vector.tensor_sub(zt, zt, MU4)
                nc.vector.tensor_mul(zt, zt, SB4)
                for b in range(B):
                    cs = slice(b * W, (b + 1) * W)
                    ro = rq_sb[:, t * B + b:t * B + b + 1]
                    if t < N_ACT:
                        nc.scalar.activation(
                            zt[:, cs], zt[:, cs],
                            mybir.ActivationFunctionType.Square,
                            accum_out=ro,
                        )
                    else:
                        # TensorScalarPtr doesn't codegen on Pool: reduces
                        # always run on VectorE
                        vsq = wsc.tile([P, W], F32, tag="vsq", name=f"vs{t}_{b}")
                        nc.vector.scalar_tensor_tensor(
                            out=vsq, in0=zt[:, cs], scalar=1.0, in1=zt[:, cs],
                            op0=mybir.AluOpType.mult,
                            op1=mybir.AluOpType.mult, accum_out=ro,
                        )
            nc.scalar.dma_start(rq_d, rq_sb)

    nc.compile()
    return nc


def kernel(z, mean, log_sd, logdet, target):
    global LAST_RESULTS, _CACHED_NC

    z = np.asarray(z, dtype=np.float32)
    mean = np.asarray(mean, dtype=np.float32)
    log_sd = np.asarray(log_sd, dtype=np.float32)
    logdet64 = np.asarray(logdet, dtype=np.float64)
    tgt = np.asarray(target).astype(np.int64)
    n, d = z.shape
    assert (n, d) == (N, D), f"kernel hardcoded for {(N, D)}, got {(n, d)}"

    # The device kernel assumes every 128-row block has the same class pattern
    # (true for the arange%2 labels). Otherwise interleave the (balanced)
    # classes host-side and un-permute logp afterwards.
    pat = tgt[:P]
    perm = None
    tgt_dev = tgt
    if not (tgt.reshape(-1, P) == pat[None, :]).all():
        idx0 = np.where(tgt == 0)[0]
        idx1 = np.where(tgt == 1)[0]
        assert len(idx0) == len(idx1), "fallback layout needs balanced classes"
        perm = np.empty(n, dtype=np.int64)
        perm[0::2] = idx0
        perm[1::2] = idx1
        z, mean, log_sd = z[perm], mean[perm], log_sd[perm]
        tgt_dev = tgt[perm]
        pat = tgt_dev[:P]

    counts = np.array([(tgt == 0).sum(), (tgt == 1).sum()], dtype=np.float64)
    patf = pat.astype(np.float32)
    oh_np = np.ascontiguousarray(np.stack([1.0 - patf, patf], axis=1))  # [P, 2]
    selT_np = np.ascontiguousarray(oh_np.T)  # [2, P]
    invc_np = (1.0 / counts).astype(np.float32).reshape(2, 1)

    if _CACHED_NC is None:
        _CACHED_NC = _build_nc()
    nc = _CACHED_NC

    bf16 = mybir.dt.np(BF16)

    def pack(x, i, dtype):
        # [N, W] slice -> [T*P, B*W]: row (t*P+p), block b at cols [b*W,(b+1)*W)
        s = x[:, i * W:(i + 1) * W].reshape(T, B, P, W)
        return np.ascontiguousarray(
            s.transpose(0, 2, 1, 3).reshape(T * P, B * W).astype(dtype)
        )

    in_maps = [
        {
            "z": pack(z, i, bf16),
            "ml": np.concatenate(
                [pack(mean, i, np.float32), pack(log_sd, i, np.float32)], axis=1
            ),
            "oh": oh_np,
            "selT": selT_np,
            "invc": invc_np,
        }
        for i in range(N_CORES)
    ]
    res = run_bass_kernel_spmd(
        nc, in_maps, core_ids=list(range(N_CORES)), trace=TRACE
    )
    LAST_RESULTS = res

    # rq[p, b] holds the partial for sample n = b*128 + p on each core
    rq = np.zeros(N, dtype=np.float64)
    for i in range(N_CORES):
        rq += np.asarray(res.results[i]["rq"]).T.reshape(-1).astype(np.float64)
    mus = np.concatenate(
        [np.asarray(res.results[i]["mus"]) for i in range(N_CORES)], axis=1
    )
    lsds = np.concatenate(
        [np.asarray(res.results[i]["lsds"]) for i in range(N_CORES)], axis=1
    )

    # logp[n] = C2[t_n] - rq[n];  C2[c] = -0.5*log(2pi)*D - sum_d lsds[c, d]
    c2 = -0.5 * LOG_2PI * D - lsds.astype(np.float64).sum(axis=1)
    logp = (c2[tgt_dev] - rq).astype(np.float32)
    if perm is not None:
        inv = np.empty_like(perm)
        inv[perm] = np.arange(n)
        logp = logp[inv]

    logp64 = logp.astype(np.float64)
    lp_tot = np.array(
        [logp64[tgt == 0].sum() / counts[0], logp64[tgt == 1].sum() / counts[1]]
    )
    ld_tot = np.array(
        [logdet64[tgt == 0].sum() / counts[0], logdet64[tgt == 1].sum() / counts[1]]
    )
    prior_logprob = np.float32((lp_tot + ld_tot).mean())
    log_p_total = lp_tot.astype(np.float32)

    return prior_logprob, mus, lsds, logp, log_p_total
